# revision 1
# baseline (speedup 1.0000x reference)
"""Trainium2 Bass kernel for nn_ConstrainedAttentionModel.

Reference semantics (B=8, T=2048, V=8192):
  emb = one_hot(x, V); x_prev = shift-right(emb)
  scores[b,t] = p0*(x[b,T-1]==x[b,t]) + p1*(t>0 and x[b,T-1]==x[b,t-1])
              + p2*(x[b,T-2]==x[b,t]) + p3*(t>0 and x[b,T-2]==x[b,t-1])
  scores[b,T-1] = -1e9
  attn = softmax(scores, axis=t)
  out[b,v] = sum_{t: x[b,t]==v} attn[b,t]

Sharding: pure data parallel, one batch row per NeuronCore (8 rows / 8 cores).

Device algorithm per core (layout t = p*16 + c, p in [0,128) partitions,
c in [0,16) chunks; token ids exact in f32):
  1. ONE packed f32 DMA PK(128,122): interleaved (c, j=5) compare block
     [x, xprev, x, xprev, t], compare values [xl,xl,xs,xs,T-1], weights
     [p0..p3,-1e9], plus host-split lo=x&63 and hi=x>>6 columns.
  2. scores in 3 wide DVE ops: M=(block==cmp) via stride-0 broadcast APs,
     M*=weights (mask folded in as the 5th slot), reduce_X -> S(128,16).
  3. E = exp(S) on ACT; row sums ES via a second ACT copy+accum op that
     stays off the P-chunk critical path.
  4. Factored one-hots in fp16 (DVE 2x mode; scalars stay f32):
     P[:,c,:] = (iota128==hi_c)*E_c  (128x128/chunk, DVE)
     AL[:,c,:] = (iota64==lo_c)      (128x64/chunk, mostly GPSIMD)
  5. out_unnorm(128,64 PSUM f32) += P_c^T-contraction AL_c, 16 chained
     fp16 matmuls (contracts t; exact one-hot values pass E through).
  6. denom bcast = ONES(128,128) @ ES -> (128,1) PSUM; RCS = 1/denom.
  7. out = out_unnorm * RCS on DVE (PSUM read), one DMA out
     (v = 64*hi + lo is row-major (128,64)).
"""

import sys

import numpy as np

if "/opt/trn_rl_repo" not in sys.path:
    sys.path.insert(0, "/opt/trn_rl_repo")

import concourse.bacc as bacc
import concourse.bass as bass
import concourse.mybir as mybir
from concourse import tile

B = 8
T = 2048
V = 8192
P = 128
C = T // P  # 16 chunks along free dim; t = p*C + c
LO = 64
NCORES = 8
NJ = 5  # compare slots: x==xl, xprev==xl, x==xs, xprev==xs, t==T-1
NPK = C * NJ + NJ + NJ + 2 * C  # compare block + cmp vals + weights + lo + hi

AL_DVE = 1  # trailing AL chunks built on DVE instead of GPSIMD

f32 = mybir.dt.float32
f16 = mybir.dt.float16
i32 = mybir.dt.int32
Alu = mybir.AluOpType
Act = mybir.ActivationFunctionType


def build_nc(reps=1, oh_dt=f16):
    nc = bacc.Bacc(None, target_bir_lowering=False)

    pk_d = nc.dram_tensor("pk", [P, NPK], f32, kind="ExternalInput")
    out_d = nc.dram_tensor("out", [V], f32, kind="ExternalOutput")

    with tile.TileContext(nc) as tc:
        with (
            tc.tile_pool(name="pool", bufs=1) as pool,
            tc.tile_pool(name="psum", bufs=1, space=bass.MemorySpace.PSUM) as psum,
        ):
          for _rep in range(reps):
              # constants first: no input deps, fills engine warm-up time
              IOT_HI = pool.tile([P, P], oh_dt, tag="IOT_HI")  # 0..127
              IOT_LO = pool.tile([P, LO], oh_dt, tag="IOT_LO")  # 0..63
              ONES = pool.tile([P, P], f32, tag="ONES")
              nc.gpsimd.iota(
                  IOT_HI[:], pattern=[[1, P]], base=0, channel_multiplier=0,
                  allow_small_or_imprecise_dtypes=True,
              )
              nc.gpsimd.iota(
                  IOT_LO[:], pattern=[[1, LO]], base=0, channel_multiplier=0,
                  allow_small_or_imprecise_dtypes=True,
              )
              nc.vector.memset(ONES[:], 1.0)

              PK = pool.tile([P, NPK], f32, tag="PK")
              nc.sync.dma_start(PK[:], pk_d[:])
              CMP3 = PK[:, 0 : C * NJ].rearrange("p (c j) -> p c j", j=NJ)
              CV = PK[:, C * NJ : C * NJ + NJ]  # [xl, xl, xs, xs, T-1]
              WT = PK[:, C * NJ + NJ : C * NJ + 2 * NJ]  # [p0..p3, -1e9]
              LOH = PK[:, C * NJ + 2 * NJ : C * NJ + 2 * NJ + C]  # lo cols
              HIH = PK[:, C * NJ + 2 * NJ + C : NPK]  # hi cols
              cv_b = bass.AP(CV.tensor, CV.offset, [CV.ap[0], [0, C], [1, NJ]])
              wt_b = bass.AP(WT.tensor, WT.offset, [WT.ap[0], [0, C], [1, NJ]])

              # AL one-hots (no E scale): mostly GPSIMD, tail chunks on DVE
              AL = pool.tile([P, C, LO], oh_dt, tag="AL")
              for c in range(C - AL_DVE):
                  nc.gpsimd.tensor_scalar(
                      AL[:, c, :], IOT_LO[:], LOH[:, c : c + 1], None, op0=Alu.is_equal
                  )

              # scores: one wide compare, one weighted mult, one j-reduce
              S = pool.tile([P, C], f32, tag="S")
              M = pool.tile([P, C, NJ], f32, tag="M")
              nc.vector.tensor_tensor(M[:], CMP3, cv_b, op=Alu.is_equal)
              nc.vector.tensor_tensor(M[:], M[:], wt_b, op=Alu.mult)
              nc.vector.tensor_reduce(
                  S[:], M[:], axis=mybir.AxisListType.X, op=Alu.add
              )

              for c in range(C - AL_DVE, C):
                  nc.vector.tensor_scalar(
                      AL[:, c, :], IOT_LO[:], LOH[:, c : c + 1], None, op0=Alu.is_equal
                  )

              # exp; row sums via a second ACT op off the critical path
              E = pool.tile([P, C], f32, tag="E")
              E2 = pool.tile([P, C], f32, tag="E2")
              ES = pool.tile([P, 1], f32, tag="ES")
              nc.scalar.activation(E[:], S[:], Act.Exp)
              nc.scalar.activation(E2[:], E[:], Act.Copy, accum_out=ES[:])

              # denominator broadcast + reciprocal (PE before the scatter chain)
              DB = psum.tile([P, 1], f32, tag="DB")
              nc.tensor.matmul(DB[:], ONES[:], ES[:], start=True, stop=True)
              RCS = pool.tile([P, 1], f32, tag="RCS")
              nc.vector.reciprocal(RCS[:], DB[:])

              # scaled hi one-hot + scatter matmuls, interleaved per chunk
              Pt = pool.tile([P, C, P], oh_dt, tag="Pt")
              OPS = psum.tile([P, LO], f32, tag="OPS")
              for c in range(C):
                  nc.vector.tensor_scalar(
                      Pt[:, c, :],
                      IOT_HI[:],
                      HIH[:, c : c + 1],
                      E[:, c : c + 1],
                      op0=Alu.is_equal,
                      op1=Alu.mult,
                  )
                  nc.tensor.matmul(
                      OPS[:], Pt[:, c, :], AL[:, c, :],
                      start=(c == 0), stop=(c == C - 1),
                  )

              # normalize on DVE (PSUM read) and write out
              O = pool.tile([P, LO], f32, tag="O")
              nc.vector.tensor_scalar(O[:], OPS[:], RCS[:], None, op0=Alu.mult)
              nc.sync.dma_start(out_d[:].rearrange("(p f) -> p f", p=P), O[:])

    nc.compile()
    return nc


_NC_CACHE = {}


def _get_nc():
    if "nc" not in _NC_CACHE:
        _NC_CACHE["nc"] = build_nc()
    return _NC_CACHE["nc"]


def make_in_maps(x, params):
    x = np.asarray(x)
    params = np.asarray(params, dtype=np.float32)
    assert x.shape == (B, T), x.shape
    in_maps = []
    tcol = np.arange(T, dtype=np.float32).reshape(P, C)
    for b in range(B):
        row = x[b].astype(np.float32)
        prev = np.empty(T, np.float32)
        prev[0] = -1.0
        prev[1:] = row[:-1]
        blk = np.empty((P, C, NJ), np.float32)
        blk[:, :, 0] = row.reshape(P, C)
        blk[:, :, 1] = prev.reshape(P, C)
        blk[:, :, 2] = row.reshape(P, C)
        blk[:, :, 3] = prev.reshape(P, C)
        blk[:, :, 4] = tcol
        pk = np.empty((P, NPK), np.float32)
        pk[:, 0 : C * NJ] = blk.reshape(P, C * NJ)
        pk[:, C * NJ : C * NJ + NJ] = np.array(
            [row[T - 1], row[T - 1], row[T - 2], row[T - 2], float(T - 1)],
            np.float32,
        )[None, :]
        pk[:, C * NJ + NJ : C * NJ + 2 * NJ] = np.array(
            [params[0], params[1], params[2], params[3], -1e9], np.float32
        )[None, :]
        xi = x[b].astype(np.int64)
        pk[:, C * NJ + 2 * NJ : C * NJ + 2 * NJ + C] = (
            (xi & 63).astype(np.float32).reshape(P, C)
        )
        pk[:, C * NJ + 2 * NJ + C : NPK] = (
            (xi >> 6).astype(np.float32).reshape(P, C)
        )
        in_maps.append({"pk": pk})
    return in_maps


def kernel(x, params):
    from concourse.bass_utils import run_bass_kernel_spmd

    nc = _get_nc()
    in_maps = make_in_maps(x, params)
    res = run_bass_kernel_spmd(nc, in_maps, list(range(NCORES)))
    out = np.stack([res.results[b]["out"] for b in range(B)], axis=0)
    return out.astype(np.float32)

